# revision 1
# baseline (speedup 1.0000x reference)
"""CIN (Compressed Interaction Network) forward kernel for 8 Trainium2 NeuronCores.

Reference computation (per batch b, embedding dim d):
    x0 = inputs[b, :, d]                 # [F=39]
    h0 = x0
    for k in 0..2:
        z  = outer(x0, h_{k})            # [F * Hk]
        h_{k+1} = z @ Wk + bk            # [256]
    out[b] = concat_k sum_d h_{k+1}      # [768]

Strategy: data-parallel over batch (64 per core).  Per core, rows r = (b, d)
are 2048 GEMM rows.  Everything is laid out transposed: x0T[f, r], hT[u, r].
The Khatri-Rao product z_T[(i,j), r] = x0T[i, r] * hT[j, r] is materialized
k-tile by k-tile on the Vector engine (fp16 -> 2x mode) from a DMA-broadcast
copy of x0T[i] and consumed immediately by the Tensor engine as the moving
operand of [K,512]-shaped matmuls accumulating into PSUM.  Weights (host
pre-cast to fp16, pre-tiled [128, KT, 256]) are the stationary operand.
The d-sum for the output is taken directly from PSUM (fp32) on the Vector
engine; the fp16 rounding of h only affects the recurrence, not the output
path.  Biases are all-zero in this model but are honored: device-side via
the ScalarE PSUM-evacuation (bias feeds the recurrence), host-side (exact)
for the D * b_k contribution to the pooled output.
"""

import os
import sys

import numpy as np

for _p in ("/opt/trn_rl_repo", "/root/.axon_site/_ro/trn_rl_repo"):
    if os.path.isdir(_p) and _p not in sys.path:
        sys.path.insert(0, _p)

N_CORES = 8
B, F, D = 512, 39, 32
U = 256
BL = B // N_CORES          # 64 batches per core
R = BL * D                 # 2048 GEMM rows per core
NB = 512                   # matmul moving free-dim (one PSUM bank of fp32)
NRB = R // NB              # 4 row blocks
K0 = F * F                 # 1521
KT0 = 13                   # layer-0 k-tiles: 3 i-values x 42 j-slots = 126 rows each
FP = 42                    # padded field count (x0 padded with 3 zero rows)
K12 = F * U                # 9984
KT12 = K12 // 128          # 78 k-tiles; kt = (i, half)

DT = "float16"             # device compute dtype for z / W / h ("float16" | "bfloat16")

_prog_cache = {}


def _np_dt():
    import ml_dtypes

    return np.float16 if DT == "float16" else ml_dtypes.bfloat16


def _build_program():
    import concourse.mybir as mybir
    from concourse import bacc, tile

    dt = mybir.dt
    cdt = getattr(dt, DT)
    f32 = dt.float32

    nc = bacc.Bacc(
        "TRN2", target_bir_lowering=False, debug=False, num_devices=N_CORES
    )
    x0_p = nc.declare_dram_parameter("x0", [FP, R], cdt, isOutput=False)
    # x0 rows each replicated 42x in DRAM: broadcast DMAs read distinct
    # addresses (HBM bank spread) instead of hammering one 4KB row.
    x0r_p = nc.declare_dram_parameter("x0r", [F * FP, R], cdt, isOutput=False)
    w0_p = nc.declare_dram_parameter("w0", [128, KT0, U], cdt, isOutput=False)
    w1_p = nc.declare_dram_parameter("w1", [128, KT12, U], cdt, isOutput=False)
    w2_p = nc.declare_dram_parameter("w2", [128, KT12, U], cdt, isOutput=False)
    bias_p = nc.declare_dram_parameter("bias", [128, 4], f32, isOutput=False)
    out_p = nc.declare_dram_parameter("out", [128, 6, BL], f32, isOutput=True)

    with tile.TileContext(nc) as tc:
        with (
            tc.tile_pool(name="const", bufs=1) as constp,
            tc.tile_pool(name="wpool", bufs=1) as wpool,
            tc.tile_pool(name="xb", bufs=5) as xbp,
            tc.tile_pool(name="zp", bufs=4) as zp,
            tc.tile_pool(name="hp", bufs=1) as hp,
            tc.tile_pool(name="psum", bufs=1, space="PSUM") as psp,
        ):
            # broadcast DMAs source from DRAM (re-reading one SBUF partition
            # 128x serializes on its port) and alternate trigger engines so
            # both dynamic HW queues run in parallel.
            bcast_n = [0]

            def bcast(dst, src_ap):
                eng = nc.sync if bcast_n[0] % 2 == 0 else nc.scalar
                bcast_n[0] += 1
                eng.dma_start(dst, src_ap)

            out_sb = constp.tile([128, 6, BL], f32, tag="out")
            h_tiles = {
                (l, c): hp.tile([128, R], cdt, tag=f"h{l}{c}", name=f"h{l}{c}")
                for l in range(2)
                for c in range(2)
            }

            # ---- prologue, hand-ordered so the critical path clears first:
            # xi[0] + xj0 head the two queues, then the first W0 k-tiles, then
            # the remaining layer-0 xi tiles interleaved with W0/W1 chunks.
            xi0_tiles = []

            def xi0_dma(kt):
                xi = xbp.tile([128, R], cdt, tag="xi", name="xi0", bufs=14)
                bcast(xi[:63, :], x0r_p[3 * kt * FP : 3 * kt * FP + 63, :])
                bcast(xi[63:126, :], x0r_p[3 * kt * FP + 63 : 3 * kt * FP + 126, :])
                xi0_tiles.append(xi)

            xj0 = constp.tile([126, R], cdt, tag="xj0")
            w0 = wpool.tile([128, KT0, U], cdt, tag="w0")
            w1 = wpool.tile([128, KT12, U], cdt, tag="w1")
            bias = constp.tile([128, 4], f32, tag="bias")

            # first-consumed tensors go in small pieces so their completion
            # semaphores fire early (DMA engines fair-share in-flight work)
            xi00 = xbp.tile([128, R], cdt, tag="xi", name="xi00", bufs=14)
            nc.sync.dma_start(xi00[:63, :], x0r_p[0:63, :])
            nc.scalar.dma_start(xj0[0:FP, :], x0_p[:, :])
            nc.sync.dma_start(xi00[63:126, :], x0r_p[63:126, :])
            nc.scalar.dma_start(xj0[FP : 2 * FP, :], x0_p[:, :])
            nc.scalar.dma_start(xj0[2 * FP : 126, :], x0_p[: 126 - 2 * FP, :])
            xi0_tiles.append(xi00)
            nc.sync.dma_start(w0[:, :2, :], w0_p[:, :2, :])
            nc.scalar.dma_start(bias[:, :], bias_p[:, :])
            xi0_dma(1)
            nc.sync.dma_start(w0[:, 2:7, :], w0_p[:, 2:7, :])
            xi0_dma(2)
            nc.scalar.dma_start(w0[:, 7:, :], w0_p[:, 7:, :])
            # only W1 chunks 0-1 load during layer 0; the rest stream in layer 1
            w1_chunks = list(range(0, KT12, 13))
            for kt in range(3, KT0):
                xi0_dma(kt)
                if kt - 3 < 1:
                    lo = w1_chunks[kt - 3]
                    (nc.sync if kt % 2 else nc.scalar).dma_start(
                        w1[:, lo : lo + 13, :], w1_p[:, lo : lo + 13, :]
                    )

            # ---- PE warm-up: the HAM clock gate needs ~3.4us of sustained
            # matmul activity to unthrottle 1.2 -> 2.4 GHz.  Startup is
            # DMA-bound anyway, so burn dummy matmuls on garbage SBUF data
            # into a PSUM bank; the first real accumulation starts with
            # start=True, which clears the bank.
            warm_ps = psp.tile([128, NB], f32, tag="ps_0_0", name="warm_ps")
            for _ in range(42):
                nc.tensor.matmul(
                    warm_ps[:, :],
                    h_tiles[(0, 0)][:, :128],
                    h_tiles[(0, 0)][:, :NB],
                    start=True,
                    stop=True,
                )

            def make_x(i, nm):
                t = xbp.tile([128, R], cdt, tag="xi", name=nm, bufs=14)
                bcast(
                    t[:, :],
                    x0r_p[i * FP : i * FP + 32, :]
                    .unsqueeze(1)
                    .to_broadcast((32, 4, R)),
                )
                return t

            l1_pre = {i: make_x(i, f"l1x{i}") for i in (0, 1)}
            l2_pre = {}

            def do_layer(l, w_t, z_fn, kt_n, kt_hook=None):
                ps = [
                    [
                        psp.tile([128, NB], f32, tag=f"ps_{c}_{r}", name=f"ps_{c}_{r}")
                        for r in range(NRB)
                    ]
                    for c in range(2)
                ]
                for kt in range(kt_n):
                    if kt_hook is not None:
                        kt_hook(kt)
                    klen, z_t = z_fn(kt)
                    for c in range(2):
                        lhsT = w_t[:klen, kt, c * 128 : (c + 1) * 128]
                        for r in range(NRB):
                            nc.tensor.matmul(
                                ps[c][r][:, :],
                                lhsT,
                                z_t[:klen, r * NB : (r + 1) * NB],
                                start=(kt == 0),
                                stop=(kt == kt_n - 1),
                            )
                # evacuations first: they gate the next layer's TTs and free the
                # PSUM banks.  The d-sum for layers 0/1 reads the fp16 h tiles
                # and is DEFERRED into the next layer's loop (kt hook) so it
                # stays off the boundary-critical DVE path.  Layer 2 has no h
                # tile, so its d-sum reads PSUM directly (no successor anyway).
                if l < 2:
                    for c in range(2):
                        for r in range(NRB):
                            # PSUM -> SBUF fp16 with per-partition bias; c=0 on
                            # DVE (same-engine gate for the next layer's first
                            # TTs), c=1 on the otherwise-idle Scalar engine so
                            # both halves evacuate in parallel at the boundary.
                            if c == 0:
                                nc.vector.tensor_scalar_add(
                                    h_tiles[(l, c)][:, r * NB : (r + 1) * NB],
                                    ps[c][r][:, :],
                                    bias[:, l * 2 + c : l * 2 + c + 1],
                                )
                            else:
                                nc.scalar.activation(
                                    h_tiles[(l, c)][:, r * NB : (r + 1) * NB],
                                    ps[c][r][:, :],
                                    mybir.ActivationFunctionType.Identity,
                                    bias=bias[:, l * 2 + c : l * 2 + c + 1],
                                )
                else:
                    for c in range(2):
                        for r in range(NRB):
                            nc.vector.tensor_reduce(
                                out_sb[:, l * 2 + c, r * (NB // D) : (r + 1) * (NB // D)],
                                ps[c][r].rearrange("p (b d) -> p b d", d=D),
                                axis=mybir.AxisListType.X,
                                op=mybir.AluOpType.add,
                            )

            def h_reduce(l):
                for c in range(2):
                    nc.vector.tensor_reduce(
                        out_sb[:, l * 2 + c, :],
                        h_tiles[(l, c)].rearrange("p (b d) -> p b d", d=D),
                        axis=mybir.AxisListType.X,
                        op=mybir.AluOpType.add,
                    )

            # ---- layer 0: k-tile t covers i in {3t, 3t+1, 3t+2} x 42 j-slots;
            # partition p = a*42 + jj; x0 rows 39..41 and the matching W0 rows
            # are zero padding, so the product is exactly 0 there. ----
            def z_layer0(kt):
                z_t = zp.tile([128, R], cdt, tag="z")
                nc.vector.tensor_mul(
                    z_t[:126, :], xi0_tiles[kt][:126, :], xj0[:126, :]
                )
                return 126, z_t

            do_layer(0, w0, z_layer0, KT0)

            # ---- layers 1, 2: z[(i, j), r] = x0[i, r] * h[j, r], k = i*256 + j ----
            def z_layer12(l, premade):
                xcur = [None]

                def fn(kt):
                    i, half = kt // 2, kt % 2
                    if half == 0:
                        if i in premade:
                            xcur[0] = premade[i]
                        else:
                            xcur[0] = make_x(i, "xi")
                    z_t = zp.tile([128, R], cdt, tag="z")
                    if kt < 2:
                        # boundary pipelining: slice-wise TT so each matmul's z
                        # slice is ready right after its h evacuation lands
                        for r in range(NRB):
                            nc.vector.tensor_mul(
                                z_t[:, r * NB : (r + 1) * NB],
                                xcur[0][:, r * NB : (r + 1) * NB],
                                h_tiles[(l - 1, half)][:, r * NB : (r + 1) * NB],
                            )
                    else:
                        nc.vector.tensor_mul(
                            z_t[:, :], xcur[0][:, :], h_tiles[(l - 1, half)][:, :]
                        )
                    return 128, z_t

                return fn

            w2 = wpool.tile([128, KT12, U], cdt, tag="w2")

            # stream the rest of W1 plus all of W2 at spread points in layer 1;
            # w1 chunk c is consumed starting at kt = 13c, w2 only in layer 2.
            w_sched = {0: (w1, w1_p, 1), 3: (w1, w1_p, 2), 8: (w1, w1_p, 3), 13: (w1, w1_p, 4),
                       20: (w1, w1_p, 5), 26: (w2, w2_p, 0), 34: (w2, w2_p, 1),
                       42: (w2, w2_p, 2), 50: (w2, w2_p, 3), 58: (w2, w2_p, 4),
                       64: (w2, w2_p, 5)}

            def w_hook(kt):
                if kt == 66:
                    l2_pre[0] = make_x(0, "l2x0")
                if kt == 70:
                    l2_pre[1] = make_x(1, "l2x1")
                if kt == 74:
                    l2_pre[2] = make_x(2, "l2x2")
                if kt == 76:
                    l2_pre[3] = make_x(3, "l2x3")
                if kt == 4:
                    h_reduce(0)   # deferred layer-0 d-sum, off the boundary path
                if kt == 6:
                    nc.sync.dma_start(out_p[:, 0:2, :], out_sb[:, 0:2, :])
                if kt in w_sched:
                    wt, wp, c = w_sched[kt]
                    lo = w1_chunks[c]
                    (nc.sync if c % 2 else nc.scalar).dma_start(
                        wt[:, lo : lo + 13, :], wp[:, lo : lo + 13, :]
                    )

            do_layer(1, w1, z_layer12(1, l1_pre), KT12, kt_hook=w_hook)

            def l2_hook(kt):
                if kt == 4:
                    h_reduce(1)   # deferred layer-1 d-sum
                if kt == 6:
                    nc.sync.dma_start(out_p[:, 2:4, :], out_sb[:, 2:4, :])

            do_layer(2, w2, z_layer12(2, l2_pre), KT12, kt_hook=l2_hook)

            nc.sync.dma_start(out_p[:, 4:6, :], out_sb[:, 4:6, :])

    nc.compile()
    return nc


def _get_program():
    if "nc" not in _prog_cache:
        _prog_cache["nc"] = _build_program()
    return _prog_cache["nc"]


def _prep_maps(inputs):
    cdt = _np_dt()
    x = np.asarray(inputs["inputs"], np.float32)          # [512, 39, 32]
    Ws = [np.asarray(inputs[f"W{k}"], np.float32) for k in range(3)]
    bs = [np.asarray(inputs[f"b{k}"], np.float32) for k in range(3)]

    # layer-0 weights: row (i, j) -> tile t = i//3, partition p = (i%3)*42 + j
    w0j = np.zeros((F, FP, U), np.float32)
    w0j[:, :F, :] = Ws[0].reshape(F, F, U)
    w0t = np.zeros((KT0, 128, U), np.float32)
    w0t[:, :126, :] = w0j.reshape(KT0, 3 * FP, U)
    w_tiled = [
        w0t.transpose(1, 0, 2).astype(cdt),
        Ws[1].reshape(KT12, 128, U).transpose(1, 0, 2).astype(cdt),
        Ws[2].reshape(KT12, 128, U).transpose(1, 0, 2).astype(cdt),
    ]
    w_tiled = [np.ascontiguousarray(w) for w in w_tiled]
    bias = np.zeros((128, 4), np.float32)
    for l in range(2):
        for c in range(2):
            bias[:, l * 2 + c] = bs[l][c * 128 : (c + 1) * 128]

    in_maps = []
    for core in range(N_CORES):
        xs = x[core * BL : (core + 1) * BL]               # [64, 39, 32]
        x0T = np.zeros((FP, R), cdt)
        x0T[:F] = xs.transpose(1, 0, 2).reshape(F, R).astype(cdt)
        x0r = np.ascontiguousarray(np.repeat(x0T[:F], FP, axis=0))
        in_maps.append(
            {
                "x0": x0T,
                "x0r": x0r,
                "w0": w_tiled[0],
                "w1": w_tiled[1],
                "w2": w_tiled[2],
                "bias": bias,
            }
        )
    return in_maps, bs


def _finish_output(results, bs):
    outs = []
    for core in range(N_CORES):
        o = np.asarray(results[core]["out"], np.float32)  # [128, 6, 64]
        outs.append(o.transpose(2, 1, 0).reshape(BL, 768))
    out = np.concatenate(outs, axis=0)
    for l in range(3):
        out[:, l * U : (l + 1) * U] += D * bs[l]
    return np.ascontiguousarray(out.astype(np.float32))


def kernel(**inputs) -> np.ndarray:
    from concourse.bass_utils import run_bass_kernel_spmd

    in_maps, bs = _prep_maps(inputs)
    nc = _get_program()
    res = run_bass_kernel_spmd(nc, in_maps, list(range(N_CORES))).results
    return _finish_output(res, bs)



# revision 37
# speedup vs baseline: 1.4819x; 1.4819x over previous
"""CIN (Compressed Interaction Network) forward kernel for 8 Trainium2 NeuronCores.

Reference computation (per batch b, embedding dim d):
    x0 = inputs[b, :, d]                 # [F=39]
    h0 = x0
    for k in 0..2:
        z  = outer(x0, h_{k})            # [F * Hk]
        h_{k+1} = z @ Wk + bk            # [256]
    out[b] = concat_k sum_d h_{k+1}      # [768]

Strategy: data-parallel over batch (64 per core).  Per core, rows r = (b, d)
are 2048 GEMM rows.  Everything is laid out transposed: x0T[f, r], hT[u, r].
The Khatri-Rao product z_T[(i,j), r] = x0T[i, r] * hT[j, r] is materialized
k-tile by k-tile on the Vector engine (fp16 -> 2x mode) from a DMA-broadcast
copy of x0T[i] and consumed immediately by the Tensor engine as the moving
operand of [K,512]-shaped matmuls accumulating into PSUM.  Weights (host
pre-cast to fp16, pre-tiled [128, KT, 256]) are the stationary operand.
The d-sum for the output is taken directly from PSUM (fp32) on the Vector
engine; the fp16 rounding of h only affects the recurrence, not the output
path.  Biases are all-zero in this model but are honored: device-side via
the ScalarE PSUM-evacuation (bias feeds the recurrence), host-side (exact)
for the D * b_k contribution to the pooled output.

Layer 2 is collapsed algebraically: the output only needs sum_d h2, and
    sum_d h2[b,u,d] = sum_{i,j} W2[(i,j),u] * G[b,i,j],
    G[b] = x0[b] @ h1[b]^T  (a per-batch F x U Gram matrix).
So instead of a [2048 x 9984 x 256] GEMM, layer 2 is: 32 PE transposes of
h1 (to d-major), 128 tiny Gram matmuls (K=32), and a [64 x 9984 x 256]
GEMM with zsum = vec(G) as the stationary operand — ~10x less PE work.
"""

import os
import sys

import numpy as np

for _p in ("/opt/trn_rl_repo", "/root/.axon_site/_ro/trn_rl_repo"):
    if os.path.isdir(_p) and _p not in sys.path:
        sys.path.insert(0, _p)

N_CORES = 8
B, F, D = 512, 39, 32
U = 256
BL = B // N_CORES          # 64 batches per core
R = BL * D                 # 2048 GEMM rows per core
NB = 512                   # matmul moving free-dim (one PSUM bank of fp32)
NRB = R // NB              # 4 row blocks
K0 = F * F                 # 1521
KT0 = 13                   # layer-0 k-tiles: 3 i-values x 42 j-slots = 126 rows each
FP = 42                    # padded field count (x0 padded with 3 zero rows)
K12 = F * U                # 9984
KT12 = K12 // 128          # 78 k-tiles; kt = (i, half)

DT = "float16"             # device compute dtype for z / W / h ("float16" | "bfloat16")

_prog_cache = {}


def _np_dt():
    import ml_dtypes

    return np.float16 if DT == "float16" else ml_dtypes.bfloat16


def _build_program():
    import concourse.mybir as mybir
    from concourse import bacc, tile
    from concourse.masks import make_identity

    dt = mybir.dt
    cdt = getattr(dt, DT)
    f32 = dt.float32

    nc = bacc.Bacc(
        "TRN2", target_bir_lowering=False, debug=False, num_devices=N_CORES
    )
    x0_p = nc.declare_dram_parameter("x0", [FP, R], cdt, isOutput=False)
    # x0 rows each replicated 42x in DRAM: broadcast DMAs read distinct
    # addresses (HBM bank spread) instead of hammering one 4KB row.
    x0r_p = nc.declare_dram_parameter("x0r", [F * FP, R], cdt, isOutput=False)
    # x0 transposed to d-major, one [128, F] slab per (b//4, b%4):
    # rows 32*(b%4)..32*(b%4)+32 hold x0[b,:,:]^T, all other rows zero, so a
    # full K=128 matmul against the 4-batch h1t chunk contracts only b's rows.
    x0d_p = nc.declare_dram_parameter(
        "x0d", [128, BL // 4, 4, F], cdt, isOutput=False
    )
    w0_p = nc.declare_dram_parameter("w0", [128, KT0, U], cdt, isOutput=False)
    w1_p = nc.declare_dram_parameter("w1", [128, KT12, U], cdt, isOutput=False)
    w2_p = nc.declare_dram_parameter("w2", [128, KT12, U], cdt, isOutput=False)
    bias_p = nc.declare_dram_parameter("bias", [128, 4], f32, isOutput=False)
    out_p = nc.declare_dram_parameter("out", [128, 4, BL], f32, isOutput=True)
    out2_p = nc.declare_dram_parameter("out2", [BL, U], f32, isOutput=True)

    with tile.TileContext(nc) as tc:
        with (
            tc.tile_pool(name="const", bufs=1) as constp,
            tc.tile_pool(name="wpool", bufs=1) as wpool,
            tc.tile_pool(name="xb", bufs=5) as xbp,
            tc.tile_pool(name="zp", bufs=4) as zp,
            tc.tile_pool(name="hp", bufs=1) as hp,
            tc.tile_pool(name="psum", bufs=1, space="PSUM") as psp,
        ):
            # broadcast DMAs source from DRAM (re-reading one SBUF partition
            # 128x serializes on its port) and alternate trigger engines so
            # both dynamic HW queues run in parallel.
            bcast_n = [0]

            def bcast(dst, src_ap):
                eng = nc.sync if bcast_n[0] % 2 == 0 else nc.scalar
                bcast_n[0] += 1
                eng.dma_start(dst, src_ap)

            out_sb = constp.tile([128, 4, BL], f32, tag="out")
            h_tiles = {
                (l, c): hp.tile([128, R], cdt, tag=f"h{l}{c}", name=f"h{l}{c}")
                for l in range(2)
                for c in range(2)
            }

            # ---- prologue, hand-ordered so the critical path clears first:
            # xi[0] + xj0 head the two queues, then the first W0 k-tiles, then
            # the remaining layer-0 xi tiles interleaved with W0/W1 chunks.
            xi0_tiles = []

            def xi0_dma(kt):
                xi = xbp.tile([128, R], cdt, tag="xi", name="xi0", bufs=14)
                bcast(xi[:63, :], x0r_p[3 * kt * FP : 3 * kt * FP + 63, :])
                bcast(xi[63:126, :], x0r_p[3 * kt * FP + 63 : 3 * kt * FP + 126, :])
                xi0_tiles.append(xi)

            xj0 = constp.tile([126, R], cdt, tag="xj0")
            w0 = wpool.tile([128, KT0, U], cdt, tag="w0")
            w1 = wpool.tile([128, KT12, U], cdt, tag="w1")
            bias = constp.tile([128, 4], f32, tag="bias")
            ident = constp.tile([128, 128], cdt, tag="ident")
            x0d_sb = constp.tile([128, BL // 4, 4, F], cdt, tag="x0d")
            make_identity(nc, ident)

            # first-consumed tensors go in small pieces so their completion
            # semaphores fire early (DMA engines fair-share in-flight work)
            xi00 = xbp.tile([128, R], cdt, tag="xi", name="xi00", bufs=14)
            nc.sync.dma_start(xi00[:63, :], x0r_p[0:63, :])
            nc.scalar.dma_start(xj0[0:FP, :], x0_p[:, :])
            nc.sync.dma_start(xi00[63:126, :], x0r_p[63:126, :])
            nc.scalar.dma_start(xj0[FP : 2 * FP, :], x0_p[:, :])
            nc.scalar.dma_start(xj0[2 * FP : 126, :], x0_p[: 126 - 2 * FP, :])
            xi0_tiles.append(xi00)
            nc.sync.dma_start(w0[:, :2, :], w0_p[:, :2, :])
            nc.scalar.dma_start(bias[:, :], bias_p[:, :])
            nc.scalar.dma_start(x0d_sb[:, :, :, :], x0d_p[:, :, :, :])
            xi0_dma(1)
            nc.sync.dma_start(w0[:, 2:7, :], w0_p[:, 2:7, :])
            xi0_dma(2)
            nc.scalar.dma_start(w0[:, 7:, :], w0_p[:, 7:, :])
            # only W1 chunks 0-1 load during layer 0; the rest stream in layer 1
            w1_chunks = list(range(0, KT12, 13))
            for kt in range(3, KT0):
                xi0_dma(kt)
                if kt - 3 < 1:
                    lo = w1_chunks[kt - 3]
                    (nc.sync if kt % 2 else nc.scalar).dma_start(
                        w1[:, lo : lo + 13, :], w1_p[:, lo : lo + 13, :]
                    )

            # ---- PE warm-up: the HAM clock gate needs ~3.4us of sustained
            # matmul activity to unthrottle 1.2 -> 2.4 GHz.  Startup is
            # DMA-bound anyway, so burn dummy matmuls on garbage SBUF data
            # into a PSUM bank; the first real accumulation starts with
            # start=True, which clears the bank.
            # warm-up source is xj0 (first DMA to land) so the warm matmuls
            # have no dependency on gpsimd init and start right after the
            # Tensor engine boots.
            warm_ps = psp.tile([128, NB], f32, tag="ps_0_0", name="warm_ps")
            for _ in range(42):
                nc.tensor.matmul(
                    warm_ps[:, :],
                    xj0[:126, :128],
                    xj0[:126, :NB],
                    start=True,
                    stop=True,
                )

            def make_x(i, nm):
                t = xbp.tile([128, R], cdt, tag="xi", name=nm, bufs=14)
                bcast(
                    t[:, :],
                    x0r_p[i * FP : i * FP + 32, :]
                    .unsqueeze(1)
                    .to_broadcast((32, 4, R)),
                )
                return t

            l1_pre = {i: make_x(i, f"l1x{i}") for i in (0, 1)}

            def do_layer(l, w_t, z_fn, kt_n, kt_hook=None):
                ps = [
                    [
                        psp.tile([128, NB], f32, tag=f"ps_{c}_{r}", name=f"ps_{c}_{r}")
                        for r in range(NRB)
                    ]
                    for c in range(2)
                ]
                for kt in range(kt_n):
                    if kt_hook is not None:
                        kt_hook(kt)
                    klen, z_t = z_fn(kt)
                    for c in range(2):
                        lhsT = w_t[:klen, kt, c * 128 : (c + 1) * 128]
                        for r in range(NRB):
                            nc.tensor.matmul(
                                ps[c][r][:, :],
                                lhsT,
                                z_t[:klen, r * NB : (r + 1) * NB],
                                start=(kt == 0),
                                stop=(kt == kt_n - 1),
                            )
                # evacuations first: they gate the next stage's consumers and
                # free the PSUM banks.  The d-sum for layers 0/1 reads the fp16
                # h tiles and is DEFERRED off the boundary-critical DVE path.
                for c in range(2):
                    for r in range(NRB):
                        # PSUM -> SBUF fp16 with per-partition bias; c=0 on
                        # DVE (same-engine gate for the next layer's first
                        # TTs), c=1 on the otherwise-idle Scalar engine so
                        # both halves evacuate in parallel at the boundary.
                        if c == 0:
                            nc.vector.tensor_scalar_add(
                                h_tiles[(l, c)][:, r * NB : (r + 1) * NB],
                                ps[c][r][:, :],
                                bias[:, l * 2 + c : l * 2 + c + 1],
                            )
                        else:
                            nc.scalar.activation(
                                h_tiles[(l, c)][:, r * NB : (r + 1) * NB],
                                ps[c][r][:, :],
                                mybir.ActivationFunctionType.Identity,
                                bias=bias[:, l * 2 + c : l * 2 + c + 1],
                            )

            def h_reduce(l):
                for c in range(2):
                    nc.vector.tensor_reduce(
                        out_sb[:, l * 2 + c, :],
                        h_tiles[(l, c)].rearrange("p (b d) -> p b d", d=D),
                        axis=mybir.AxisListType.X,
                        op=mybir.AluOpType.add,
                    )

            # ---- layer 0: k-tile t covers i in {3t, 3t+1, 3t+2} x 42 j-slots;
            # partition p = a*42 + jj; x0 rows 39..41 and the matching W0 rows
            # are zero padding, so the product is exactly 0 there. ----
            def z_layer0(kt):
                z_t = zp.tile([128, R], cdt, tag="z")
                nc.vector.tensor_mul(
                    z_t[:126, :], xi0_tiles[kt][:126, :], xj0[:126, :]
                )
                return 126, z_t

            do_layer(0, w0, z_layer0, KT0)

            # ---- layers 1, 2: z[(i, j), r] = x0[i, r] * h[j, r], k = i*256 + j ----
            def z_layer12(l, premade):
                xcur = [None]

                def fn(kt):
                    i, half = kt // 2, kt % 2
                    if half == 0:
                        if i in premade:
                            xcur[0] = premade[i]
                        else:
                            xcur[0] = make_x(i, "xi")
                    z_t = zp.tile([128, R], cdt, tag="z")
                    if kt < 2:
                        # boundary pipelining: slice-wise TT so each matmul's z
                        # slice is ready right after its h evacuation lands
                        for r in range(NRB):
                            nc.vector.tensor_mul(
                                z_t[:, r * NB : (r + 1) * NB],
                                xcur[0][:, r * NB : (r + 1) * NB],
                                h_tiles[(l - 1, half)][:, r * NB : (r + 1) * NB],
                            )
                    else:
                        nc.vector.tensor_mul(
                            z_t[:, :], xcur[0][:, :], h_tiles[(l - 1, half)][:, :]
                        )
                    return 128, z_t

                return fn

            w2 = wpool.tile([128, KT12, U], cdt, tag="w2")

            # stream the rest of W1 plus all of W2 at spread points in layer 1;
            # w1 chunk c is consumed starting at kt = 13c, w2 only in layer 2.
            w_sched = {0: (w1, w1_p, 1), 3: (w1, w1_p, 2), 8: (w1, w1_p, 3), 13: (w1, w1_p, 4),
                       20: (w1, w1_p, 5), 26: (w2, w2_p, 0), 34: (w2, w2_p, 1),
                       42: (w2, w2_p, 2), 50: (w2, w2_p, 3), 58: (w2, w2_p, 4),
                       64: (w2, w2_p, 5)}

            def w_hook(kt):
                if kt == 4:
                    h_reduce(0)   # deferred layer-0 d-sum, off the boundary path
                if kt == 6:
                    nc.sync.dma_start(out_p[:, 0:2, :], out_sb[:, 0:2, :])
                if kt in w_sched:
                    wt, wp, c = w_sched[kt]
                    lo = w1_chunks[c]
                    (nc.sync if c % 2 else nc.scalar).dma_start(
                        wt[:, lo : lo + 13, :], wp[:, lo : lo + 13, :]
                    )

            do_layer(1, w1, z_layer12(1, l1_pre), KT12, kt_hook=w_hook)

            # ---- layer 2, collapsed.  (1) PE-transpose h1 to d-major in
            # 128-row chunks (4 batches each); (2) per (b, j-half) Gram
            # matmuls G[b][j, i] with K=32 (the d contraction); (3) one
            # [64 x 256] GEMM accumulating all 78 W2 k-tiles, zsum stationary.
            h1t = constp.tile([128, NRB * 4, U], cdt, tag="h1t")
            zsum = constp.tile([128, F, 2, BL], cdt, tag="zsum")
            NRC = R // 128  # 16 transpose chunks
            def emit_t(rc):
                for c in range(2):
                    pst = psp.tile([128, 128], cdt, tag=f"ps_{c}_{rc % 2}", name="pst")
                    nc.tensor.transpose(
                        pst[:, :],
                        h_tiles[(1, c)][:, rc * 128 : (rc + 1) * 128],
                        ident[:, :],
                    )
                    nc.vector.tensor_copy(
                        out=h1t[:, rc, c * 128 : (c + 1) * 128], in_=pst[:, :]
                    )

            GPTAGS = ["ps_0_2", "ps_1_2", "ps_1_3"]

            def emit_g(rc):
                gp = {
                    c: psp.tile(
                        [128, 4 * 64], f32, tag=GPTAGS[(2 * rc + c) % 3], name="gp"
                    )
                    for c in range(2)
                }
                for bb in range(4):
                    for c in range(2):
                        nc.tensor.matmul(
                            gp[c][:, bb * 64 : bb * 64 + F],
                            h1t[:, rc, c * 128 : (c + 1) * 128],
                            x0d_sb[:, rc, bb, :],
                            start=True,
                            stop=True,
                        )
                for c in range(2):
                    # zsum[(i,j), b]: k-tile t = 2i + c, partition p = j % 128
                    nc.vector.tensor_copy(
                        out=zsum[:, :, c, rc * 4 : rc * 4 + 4],
                        in_=gp[c].rearrange("p (b i) -> p i b", i=64)[:, :F, :],
                    )

            # transposes run one chunk ahead of the Gram matmuls so each
            # chunk's h1t evacuation (DVE) hides under the previous chunk's
            # G matmuls (PE)
            emit_t(0)
            for rc in range(1, NRC):
                emit_t(rc)
                emit_g(rc - 1)
            emit_g(NRC - 1)

            h_reduce(1)   # d-sum on DVE, overlaps the final GEMM below
            nc.sync.dma_start(out_p[:, 2:4, :], out_sb[:, 2:4, :])

            ps2 = psp.tile([BL, U], f32, tag="ps_0_3", name="ps2")
            out2_sb = constp.tile([BL, U], f32, tag="out2")
            for t in range(KT12):
                i, c = t // 2, t % 2
                nc.tensor.matmul(
                    ps2[:, :],
                    zsum[:, i, c, :],
                    w2[:, t, :],
                    start=(t == 0),
                    stop=(t == KT12 - 1),
                )
            nc.vector.tensor_copy(out=out2_sb[:, :], in_=ps2[:, :])
            nc.sync.dma_start(out2_p[:, :], out2_sb[:, :])

    nc.compile()
    return nc


def _get_program():
    if "nc" not in _prog_cache:
        _prog_cache["nc"] = _build_program()
    return _prog_cache["nc"]


def _prep_maps(inputs):
    cdt = _np_dt()
    x = np.asarray(inputs["inputs"], np.float32)          # [512, 39, 32]
    Ws = [np.asarray(inputs[f"W{k}"], np.float32) for k in range(3)]
    bs = [np.asarray(inputs[f"b{k}"], np.float32) for k in range(3)]

    # layer-0 weights: row (i, j) -> tile t = i//3, partition p = (i%3)*42 + j
    w0j = np.zeros((F, FP, U), np.float32)
    w0j[:, :F, :] = Ws[0].reshape(F, F, U)
    w0t = np.zeros((KT0, 128, U), np.float32)
    w0t[:, :126, :] = w0j.reshape(KT0, 3 * FP, U)
    w_tiled = [
        w0t.transpose(1, 0, 2).astype(cdt),
        Ws[1].reshape(KT12, 128, U).transpose(1, 0, 2).astype(cdt),
        Ws[2].reshape(KT12, 128, U).transpose(1, 0, 2).astype(cdt),
    ]
    w_tiled = [np.ascontiguousarray(w) for w in w_tiled]
    bias = np.zeros((128, 4), np.float32)
    for l in range(2):
        for c in range(2):
            bias[:, l * 2 + c] = bs[l][c * 128 : (c + 1) * 128]

    in_maps = []
    for core in range(N_CORES):
        xs = x[core * BL : (core + 1) * BL]               # [64, 39, 32]
        x0T = np.zeros((FP, R), cdt)
        x0T[:F] = xs.transpose(1, 0, 2).reshape(F, R).astype(cdt)
        x0r = np.ascontiguousarray(np.repeat(x0T[:F], FP, axis=0))
        # [128, 16, 4, 39]: slab (rc, bb) has x0[rc*4+bb]^T in rows
        # 32*bb..32*bb+32, zeros elsewhere
        x0d = np.zeros((128, 16, 4, F), np.float32)
        xsT = xs.reshape(16, 4, F, D).transpose(0, 1, 3, 2)  # [16, 4, 32, 39]
        for bb in range(4):
            x0d[32 * bb : 32 * (bb + 1), :, bb, :] = xsT[:, bb].transpose(1, 0, 2)
        x0d = np.ascontiguousarray(x0d.astype(cdt))
        in_maps.append(
            {
                "x0": x0T,
                "x0r": x0r,
                "x0d": x0d,
                "w0": w_tiled[0],
                "w1": w_tiled[1],
                "w2": w_tiled[2],
                "bias": bias,
            }
        )
    return in_maps, bs


def _finish_output(results, bs):
    outs = []
    for core in range(N_CORES):
        o = np.asarray(results[core]["out"], np.float32)  # [128, 4, 64]
        o2 = np.asarray(results[core]["out2"], np.float32)  # [64, 256]
        outs.append(
            np.concatenate([o.transpose(2, 1, 0).reshape(BL, 2 * U), o2], axis=1)
        )
    out = np.concatenate(outs, axis=0)
    for l in range(3):
        out[:, l * U : (l + 1) * U] += D * bs[l]
    return np.ascontiguousarray(out.astype(np.float32))


def kernel(**inputs) -> np.ndarray:
    from concourse.bass_utils import run_bass_kernel_spmd

    in_maps, bs = _prep_maps(inputs)
    nc = _get_program()
    res = run_bass_kernel_spmd(nc, in_maps, list(range(N_CORES))).results
    return _finish_output(res, bs)

